# revision 12
# baseline (speedup 1.0000x reference)
"""Trainium2 Bass kernel for nn_MixChan (dense_mlp).

Reference computation (per batch sample b):
    d   = dist / dist.sum()                       # (32,)
    xs  = x.sum(axis=K) * d[c]                    # (32, 512, 512)
    ds  = avgpool4x4(xs)                          # (32, 128, 128)
    h1  = leaky_relu(ds.flat @ W1.T + b1, 0.2)    # (32, 1024)
    coef= leaky_relu(h1 @ W2.T + b2, 0.2)         # (32, 1)
    out = einsum('c,cwh->wh', coef, xs) / 32      # (512, 512)

Sharding: data-parallel over batch B=8 across the 8 NeuronCores; the MLP
weights are replicated (W1 is streamed from HBM as bf16).

Per-core dataflow, one pass over x in 32 tiles of 16 image rows each.
SBUF layout puts partition p = (c, rb) with rb the 4-row pool group, so
every DMA descriptor is 4 contiguous image rows (8 KiB) and the DRAM-side
outermost dim is c=32, which spreads each transfer across all 16 SDMA
engines (the previous layout's outermost dim of 4 left 12 engines idle
and capped the x load at ~90 GB/s).

  per tile t (rows 16t..16t+15, all 32 channels, both k):
    - DMA xt [128p=(c,rb), (k2, r4, h512)] f32           (2 MiB, 8 KiB descs)
    - DMA w1c [128p=j, (rb4, h1024)] bf16                (1 MiB)
    - DVE: k-sum -> xs slab slice [128, 2048] bf16       (slab kept, 16 MiB)
    - DVE: 4x4-pool XY-reduce -> pool [128p=(c,rb), j128] f32
    - PE : pool.T @ dmat -> ptps [j, (c,rb)] PSUM        (dmat = diag(d/16))
    - ACT: copy/cast + reorder -> ptsb [j, (rb,c)] bf16
    - PE : h1[c,:] += ptsb[:,rb].T @ w1c[:,rb]           (PSUM accum, x4 rb)
  MLP tail (tiny): bias+leaky, dot W2, leaky, S[p,m] = u_c*delta(rb,m)
  per tile t: out rows = S.T @ slab slice (PE), PSUM->SBUF copy, DMA out
"""

import numpy as np
import ml_dtypes

B, C, K, W = 8, 32, 2, 512
P = 4                 # pool kernel/stride
G = W // P            # 128 pooled rows/cols
FEAT = G * G          # 16384
HID = 1024
NEG = 0.2
N_CORES = 8
T = 32                # x tiles (16 image rows each)
RB = 4                # pool row-groups per tile
TF = K * P * W        # xt free size per k... (k2, r4, h512) = 4096 total
SL = P * W            # slab slice free size per tile (r4, h512) = 2048

_prog_cache = {}


def _build_program():
    import concourse.bass as bass
    import concourse.tile as tile
    from concourse import bacc, mybir

    dt = mybir.dt
    f32 = dt.float32
    bf16 = dt.bfloat16
    Alu = mybir.AluOpType
    AX = mybir.AxisListType

    nc = bacc.Bacc(
        "TRN2",
        debug=False,
        enable_asserts=False,
        target_bir_lowering=False,
        num_devices=N_CORES,
    )

    x_t = nc.dram_tensor("x", [C, K, W, W], f32, kind="ExternalInput").ap()
    w1t_t = nc.dram_tensor("w1t", [FEAT, HID], bf16, kind="ExternalInput").ap()
    dmat_t = nc.dram_tensor("dmat", [128, 128], f32, kind="ExternalInput").ap()
    m0_t = nc.dram_tensor("m0", [128, P], bf16, kind="ExternalInput").ap()
    t128_t = nc.dram_tensor("t128", [128, 128], f32, kind="ExternalInput").ap()
    b1r_t = nc.dram_tensor("b1r", [C, HID], f32, kind="ExternalInput").ap()
    w2r_t = nc.dram_tensor("w2r", [C, HID], f32, kind="ExternalInput").ap()
    b2r_t = nc.dram_tensor("b2r", [C, 1], f32, kind="ExternalInput").ap()
    out_t = nc.dram_tensor("out", [W, W], f32, kind="ExternalOutput").ap()

    with tile.TileContext(nc) as tc:
        with (
            tc.tile_pool(name="singles", bufs=1) as singles,
            tc.tile_pool(name="small", bufs=1) as small,
        ):
            # constants
            dmat_sb = singles.tile([128, 128], f32)
            nc.sync.dma_start(dmat_sb[:], dmat_t)
            m0_sb = singles.tile([128, P], bf16)
            nc.sync.dma_start(m0_sb[:], m0_t)
            t128_sb = singles.tile([128, 128], f32)
            nc.sync.dma_start(t128_sb[:], t128_t)
            w2r_sb = singles.tile([C, HID], f32)
            nc.sync.dma_start(w2r_sb[:], w2r_t)
            b2r_sb = singles.tile([C, 1], f32)
            nc.sync.dma_start(b2r_sb[:], b2r_t)

            # xs slab: k-summed (unscaled) x, bf16, [(c32 rb4), (t32 r4 h512)]
            xs_sb = singles.tile([128, T * SL], bf16)
            S_sb = singles.tile([128, P], bf16)

            b1r_sb = singles.tile([C, HID], f32)
            nc.sync.dma_start(b1r_sb[:], b1r_t)

            with tc.tile_pool(name="psum_h1", bufs=1, space="PSUM") as ph1:
                h1p = ph1.tile([C, HID], f32)

                with (
                    tc.tile_pool(name="xt", bufs=2) as xtp,
                    tc.tile_pool(name="w1c", bufs=2) as w1p,
                    tc.tile_pool(name="pool", bufs=2) as poolp,
                    tc.tile_pool(name="ptsb", bufs=2) as ptp,
                    tc.tile_pool(name="psum_pt", bufs=2, space="PSUM") as pptp,
                ):
                    for t in range(T):
                        xt = xtp.tile([128, K, P * W], f32)
                        # src iterates (c, rb, 4rows*512): descriptors are
                        # 4 contiguous image rows = 8 KiB; outermost dim c=32
                        # spreads the transfer over all 16 SDMA engines.
                        for k in range(K):
                            xg = x_t[:, k, 16 * t : 16 * (t + 1), :].rearrange(
                                "c (rb r) h -> c rb (r h)", rb=RB
                            )
                            nc.sync.dma_start(xt[:, k, :], xg)

                        w1c = w1p.tile([128, RB, HID], bf16)
                        w1g = w1t_t[512 * t : 512 * (t + 1), :].rearrange(
                            "(rb j) h -> j rb h", rb=RB
                        )
                        nc.sync.dma_start(w1c[:], w1g)

                        xs_slice = xs_sb[:, SL * t : SL * (t + 1)]
                        nc.vector.tensor_add(xs_slice, xt[:, 0, :], xt[:, 1, :])

                        # 4x4 avgpool (sum; /16 and d_c fold into dmat):
                        # reduce over (r, f) keeping j -> [128, 128]
                        pool = poolp.tile([128, G], f32)
                        nc.vector.reduce_sum(
                            pool[:],
                            xs_slice.rearrange("p (r j f) -> p j r f", r=P, f=P),
                            axis=AX.XY,
                        )

                        # transpose + scale: ptps[j, (c,rb)] = pool[(c,rb), j]*d_c/16
                        ptps = pptp.tile([128, 128], f32)
                        nc.tensor.matmul(
                            ptps[:], lhsT=pool[:], rhs=dmat_sb[:],
                            start=True, stop=True,
                        )
                        # cast to bf16, reordering free dim (c,rb) -> (rb,c)
                        ptsb = ptp.tile([128, 128], bf16)
                        nc.scalar.copy(
                            ptsb[:].rearrange("j (rb c) -> j rb c", rb=RB),
                            ptps[:].rearrange("j (c rb) -> j rb c", rb=RB),
                        )

                        for rb in range(RB):
                            first = t == 0 and rb == 0
                            last = t == T - 1 and rb == RB - 1
                            lhs = ptsb[:, 32 * rb : 32 * (rb + 1)]
                            nc.tensor.matmul(
                                h1p[:, 0:512], lhsT=lhs, rhs=w1c[:, rb, 0:512],
                                start=first, stop=last,
                            )
                            nc.tensor.matmul(
                                h1p[:, 512:1024], lhsT=lhs,
                                rhs=w1c[:, rb, 512:1024],
                                start=first, stop=last,
                            )

                # ---- MLP tail ----
                tmp1 = small.tile([C, HID], f32)
                nc.vector.tensor_add(tmp1[:], h1p[:], b1r_sb[:])
                h1s = small.tile([C, HID], f32)
                nc.vector.scalar_tensor_tensor(
                    out=h1s[:], in0=tmp1[:], scalar=NEG, in1=tmp1[:],
                    op0=Alu.mult, op1=Alu.max,
                )
                # coef = leaky(h1s @ W2.T + b2): tensor_tensor_reduce crashes
                # the exec unit on HW, so mul + reduce + add instead.
                prod = small.tile([C, HID], f32)
                nc.vector.tensor_mul(prod[:], h1s[:], w2r_sb[:])
                red = small.tile([C, 1], f32)
                nc.vector.reduce_sum(red[:], prod[:], axis=AX.X)
                cf = small.tile([C, 1], f32)
                nc.vector.tensor_add(cf[:], red[:], b2r_sb[:])
                # coef padded to 128 partitions for a K=128 matmul (small-K
                # matmuls are flaky on HW).
                coef128 = small.tile([128, 1], f32)
                nc.vector.memset(coef128[:], 0.0)
                nc.vector.scalar_tensor_tensor(
                    out=coef128[0:C, :], in0=cf[:], scalar=NEG, in1=cf[:],
                    op0=Alu.mult, op1=Alu.max,
                )
                with tc.tile_pool(name="psum_u", bufs=1, space="PSUM") as pup:
                    u_ps = pup.tile([128, 1], f32)
                    # u128[p=(c,rb)] = coef[c] * d[c] / 32  (t128 folds d/32)
                    nc.tensor.matmul(
                        u_ps[:], lhsT=t128_sb[:], rhs=coef128[:],
                        start=True, stop=True,
                    )
                    nc.vector.tensor_scalar_mul(S_sb[:], m0_sb[:], u_ps[:])

            # ---- weighted channel sum ----
            with (
                tc.tile_pool(name="psum_o", bufs=2, space="PSUM") as pop,
                tc.tile_pool(name="ob", bufs=3) as obp,
            ):
                for t in range(T):
                    po = pop.tile([P, SL], f32)
                    for i in range(4):
                        nc.tensor.matmul(
                            po[:, 512 * i : 512 * (i + 1)],
                            lhsT=S_sb[:],
                            rhs=xs_sb[:, SL * t + 512 * i : SL * t + 512 * (i + 1)],
                            start=True, stop=True,
                        )
                    ob = obp.tile([P, SL], f32)
                    nc.vector.tensor_copy(ob[:, 0:1024], po[:, 0:1024])
                    nc.scalar.copy(ob[:, 1024:2048], po[:, 1024:2048])
                    og = out_t[16 * t : 16 * (t + 1), :].rearrange(
                        "(rb r) h -> rb (r h)", rb=RB
                    )
                    nc.sync.dma_start(og, ob[:])

    nc.compile()
    return nc


def _get_program():
    if "nc" not in _prog_cache:
        _prog_cache["nc"] = _build_program()
    return _prog_cache["nc"]


def prep_in_maps(x, dist, W1, b1, W2, b2):
    bf16 = ml_dtypes.bfloat16
    x = np.asarray(x, dtype=np.float32)
    dist = np.asarray(dist, dtype=np.float32)
    W1 = np.asarray(W1, dtype=np.float32)
    b1 = np.asarray(b1, dtype=np.float32)
    W2 = np.asarray(W2, dtype=np.float32)
    b2 = np.asarray(b2, dtype=np.float32)

    d = dist / dist.sum()
    dr = np.repeat(d, RB)  # d[p//4] for p=(c,rb)
    # dmat[p, q] = delta(p, q) * d[q//4] / 16: transpose + pool-norm + d
    dmat = np.diag(dr / (P * P)).astype(np.float32)
    # m0[p, m] = delta(p % 4, m)
    m0 = np.tile(np.eye(P, dtype=np.float32), (C, 1)).astype(bf16)
    # t128[p, q] = delta(p, q//4) * d[q//4] / 32 for p < C
    t128 = np.zeros((128, 128), np.float32)
    t128[np.arange(128) // RB, np.arange(128)] = dr / C
    b1r = np.ascontiguousarray(np.broadcast_to(b1, (C, HID))).astype(np.float32)
    w2r = np.ascontiguousarray(np.broadcast_to(W2[0], (C, HID))).astype(np.float32)
    b2r = np.full((C, 1), b2[0], dtype=np.float32)
    w1t = np.ascontiguousarray(W1.T).astype(bf16)

    return [
        dict(
            x=np.ascontiguousarray(x[b]),
            w1t=w1t,
            dmat=dmat,
            m0=m0,
            t128=t128,
            b1r=b1r,
            w2r=w2r,
            b2r=b2r,
        )
        for b in range(N_CORES)
    ]


def kernel(x, dist, W1, b1, W2, b2):
    from concourse.bass_utils import run_bass_kernel_spmd

    in_maps = prep_in_maps(x, dist, W1, b1, W2, b2)
    nc = _get_program()
    res = run_bass_kernel_spmd(nc, in_maps, list(range(N_CORES)))
    out = np.stack([res.results[i]["out"] for i in range(N_CORES)])
    return out[:, None, :, :].astype(np.float32)


# revision 13
# speedup vs baseline: 1.0513x; 1.0513x over previous
"""Trainium2 Bass kernel for nn_MixChan (dense_mlp).

Reference computation (per batch sample b):
    d   = dist / dist.sum()                       # (32,)
    xs  = x.sum(axis=K) * d[c]                    # (32, 512, 512)
    ds  = avgpool4x4(xs)                          # (32, 128, 128)
    h1  = leaky_relu(ds.flat @ W1.T + b1, 0.2)    # (32, 1024)
    coef= leaky_relu(h1 @ W2.T + b2, 0.2)         # (32, 1)
    out = einsum('c,cwh->wh', coef, xs) / 32      # (512, 512)

Sharding: data-parallel over batch B=8 across the 8 NeuronCores; the MLP
weights are replicated (W1 is streamed from HBM as bf16).

Per-core dataflow, one pass over x in 32 tiles of 16 image rows each.
SBUF layout puts partition p = (c, rb) with rb the 4-row pool group, so
every DMA descriptor is 4 contiguous image rows (8 KiB) and the DRAM-side
outermost dim is c=32, which spreads each transfer across all 16 SDMA
engines (the previous layout's outermost dim of 4 left 12 engines idle
and capped the x load at ~90 GB/s).

  per tile t (rows 16t..16t+15, all 32 channels, both k):
    - DMA xt [128p=(c,rb), (k2, r4, h512)] f32           (2 MiB, 8 KiB descs)
    - DMA w1c [128p=j, (rb4, h1024)] bf16                (1 MiB)
    - DVE: k-sum -> xs slab slice [128, 2048] bf16       (slab kept, 16 MiB)
    - DVE: 4x4-pool XY-reduce -> pool [128p=(c,rb), j128] f32
    - PE : pool.T @ dmat -> ptps [j, (c,rb)] PSUM        (dmat = diag(d/16))
    - ACT: copy/cast + reorder -> ptsb [j, (rb,c)] bf16
    - PE : h1[c,:] += ptsb[:,rb].T @ w1c[:,rb]           (PSUM accum, x4 rb)
  MLP tail (tiny): bias+leaky, dot W2, leaky, S[p,m] = u_c*delta(rb,m)
  per tile t: out rows = S.T @ slab slice (PE), PSUM->SBUF copy, DMA out
"""

import numpy as np
import ml_dtypes

B, C, K, W = 8, 32, 2, 512
P = 4                 # pool kernel/stride
G = W // P            # 128 pooled rows/cols
FEAT = G * G          # 16384
HID = 1024
NEG = 0.2
N_CORES = 8
T = 32                # x tiles (16 image rows each)
RB = 4                # pool row-groups per tile
TF = K * P * W        # xt free size per k... (k2, r4, h512) = 4096 total
SL = P * W            # slab slice free size per tile (r4, h512) = 2048

_prog_cache = {}


def _build_program():
    import concourse.bass as bass
    import concourse.tile as tile
    from concourse import bacc, mybir

    dt = mybir.dt
    f32 = dt.float32
    bf16 = dt.bfloat16
    Alu = mybir.AluOpType
    AX = mybir.AxisListType

    nc = bacc.Bacc(
        "TRN2",
        debug=False,
        enable_asserts=False,
        target_bir_lowering=False,
        num_devices=N_CORES,
    )

    x_t = nc.dram_tensor("x", [C, K, W, W], f32, kind="ExternalInput").ap()
    w1t_t = nc.dram_tensor("w1t", [FEAT, HID], bf16, kind="ExternalInput").ap()
    dmat_t = nc.dram_tensor("dmat", [128, 128], f32, kind="ExternalInput").ap()
    m0_t = nc.dram_tensor("m0", [128, P], bf16, kind="ExternalInput").ap()
    t128_t = nc.dram_tensor("t128", [128, 128], f32, kind="ExternalInput").ap()
    b1r_t = nc.dram_tensor("b1r", [C, HID], f32, kind="ExternalInput").ap()
    w2r_t = nc.dram_tensor("w2r", [C, HID], f32, kind="ExternalInput").ap()
    b2r_t = nc.dram_tensor("b2r", [C, 1], f32, kind="ExternalInput").ap()
    out_t = nc.dram_tensor("out", [W, W], f32, kind="ExternalOutput").ap()

    with tile.TileContext(nc) as tc:
        with (
            tc.tile_pool(name="singles", bufs=1) as singles,
            tc.tile_pool(name="small", bufs=1) as small,
        ):
            # constants
            dmat_sb = singles.tile([128, 128], f32)
            nc.sync.dma_start(dmat_sb[:], dmat_t)
            m0_sb = singles.tile([128, P], bf16)
            nc.sync.dma_start(m0_sb[:], m0_t)
            t128_sb = singles.tile([128, 128], f32)
            nc.sync.dma_start(t128_sb[:], t128_t)
            w2r_sb = singles.tile([C, HID], f32)
            nc.sync.dma_start(w2r_sb[:], w2r_t)
            b2r_sb = singles.tile([C, 1], f32)
            nc.sync.dma_start(b2r_sb[:], b2r_t)

            # xs slab: k-summed (unscaled) x, bf16, [(c32 rb4), (t32 r4 h512)]
            xs_sb = singles.tile([128, T * SL], bf16)
            S_sb = singles.tile([128, P], bf16)

            b1r_sb = singles.tile([C, HID], f32)
            nc.sync.dma_start(b1r_sb[:], b1r_t)

            with tc.tile_pool(name="psum_h1", bufs=1, space="PSUM") as ph1:
                h1p = ph1.tile([C, HID], f32)

                with (
                    tc.tile_pool(name="xt", bufs=2) as xtp,
                    tc.tile_pool(name="w1c", bufs=2) as w1p,
                    tc.tile_pool(name="pool", bufs=2) as poolp,
                    tc.tile_pool(name="ptsb", bufs=2) as ptp,
                    tc.tile_pool(name="psum_pt", bufs=2, space="PSUM") as pptp,
                ):
                    for t in range(T):
                        xt = xtp.tile([128, K, P * W], f32)
                        # src iterates (c, rb, 4rows*512): descriptors are
                        # 4 contiguous image rows = 8 KiB; outermost dim c=32
                        # spreads the transfer over all 16 SDMA engines.
                        for k in range(K):
                            xg = x_t[:, k, 16 * t : 16 * (t + 1), :].rearrange(
                                "c (rb r) h -> c rb (r h)", rb=RB
                            )
                            nc.sync.dma_start(xt[:, k, :], xg)

                        w1c = w1p.tile([128, RB, HID], bf16)
                        w1g = w1t_t[512 * t : 512 * (t + 1), :].rearrange(
                            "(rb j) h -> j rb h", rb=RB
                        )
                        nc.sync.dma_start(w1c[:], w1g)

                        xs_slice = xs_sb[:, SL * t : SL * (t + 1)]
                        nc.vector.tensor_add(xs_slice, xt[:, 0, :], xt[:, 1, :])

                        # 4x4 avgpool (sum; /16 and d_c fold into dmat):
                        # reduce over (r, f) keeping j -> [128, 128]
                        pool = poolp.tile([128, G], f32)
                        nc.vector.reduce_sum(
                            pool[:],
                            xs_slice.rearrange("p (r j f) -> p j r f", r=P, f=P),
                            axis=AX.XY,
                        )

                        # transpose + scale: ptps[j, (c,rb)] = pool[(c,rb), j]*d_c/16
                        ptps = pptp.tile([128, 128], f32)
                        nc.tensor.matmul(
                            ptps[:], lhsT=pool[:], rhs=dmat_sb[:],
                            start=True, stop=True,
                        )
                        # cast to bf16, reordering free dim (c,rb) -> (rb,c)
                        ptsb = ptp.tile([128, 128], bf16)
                        nc.scalar.copy(
                            ptsb[:].rearrange("j (rb c) -> j rb c", rb=RB),
                            ptps[:].rearrange("j (c rb) -> j rb c", rb=RB),
                        )

                        for rb in range(RB):
                            first = t == 0 and rb == 0
                            last = t == T - 1 and rb == RB - 1
                            lhs = ptsb[:, 32 * rb : 32 * (rb + 1)]
                            nc.tensor.matmul(
                                h1p[:, 0:512], lhsT=lhs, rhs=w1c[:, rb, 0:512],
                                start=first, stop=last,
                            )
                            nc.tensor.matmul(
                                h1p[:, 512:1024], lhsT=lhs,
                                rhs=w1c[:, rb, 512:1024],
                                start=first, stop=last,
                            )

                # ---- MLP tail ----
                tmp1 = small.tile([C, HID], f32)
                nc.vector.tensor_add(tmp1[:], h1p[:], b1r_sb[:])
                h1s = small.tile([C, HID], f32)
                nc.vector.scalar_tensor_tensor(
                    out=h1s[:], in0=tmp1[:], scalar=NEG, in1=tmp1[:],
                    op0=Alu.mult, op1=Alu.max,
                )
                # coef = leaky(h1s @ W2.T + b2): tensor_tensor_reduce crashes
                # the exec unit on HW, so mul + reduce + add instead.
                prod = small.tile([C, HID], f32)
                nc.vector.tensor_mul(prod[:], h1s[:], w2r_sb[:])
                red = small.tile([C, 1], f32)
                nc.vector.reduce_sum(red[:], prod[:], axis=AX.X)
                cf = small.tile([C, 1], f32)
                nc.vector.tensor_add(cf[:], red[:], b2r_sb[:])
                # coef padded to 128 partitions for a K=128 matmul (small-K
                # matmuls are flaky on HW).
                coef128 = small.tile([128, 1], f32)
                nc.vector.memset(coef128[:], 0.0)
                nc.vector.scalar_tensor_tensor(
                    out=coef128[0:C, :], in0=cf[:], scalar=NEG, in1=cf[:],
                    op0=Alu.mult, op1=Alu.max,
                )
                with tc.tile_pool(name="psum_u", bufs=1, space="PSUM") as pup:
                    u_ps = pup.tile([128, 1], f32)
                    # u128[p=(c,rb)] = coef[c] * d[c] / 32  (t128 folds d/32)
                    nc.tensor.matmul(
                        u_ps[:], lhsT=t128_sb[:], rhs=coef128[:],
                        start=True, stop=True,
                    )
                    nc.vector.tensor_scalar_mul(S_sb[:], m0_sb[:], u_ps[:])

            # ---- weighted channel sum ----
            with (
                tc.tile_pool(name="psum_o", bufs=8, space="PSUM") as pop,
                tc.tile_pool(name="ob", bufs=4) as obp,
            ):
                # quarter-granular: 8 single-bank PSUM bufs give the PE two
                # tiles of lookahead over the PSUM->SBUF copies, which
                # alternate between DVE and ACT per 512-col quarter.
                for t in range(T):
                    og = out_t[16 * t : 16 * (t + 1), :].rearrange(
                        "(rb r) h -> rb (r h)", rb=RB
                    )
                    ob = obp.tile([P, SL], f32)
                    for i in range(4):
                        po = pop.tile([P, 512], f32)
                        nc.tensor.matmul(
                            po[:],
                            lhsT=S_sb[:],
                            rhs=xs_sb[:, SL * t + 512 * i : SL * t + 512 * (i + 1)],
                            start=True, stop=True,
                        )
                        dst = ob[:, 512 * i : 512 * (i + 1)]
                        if i % 2 == 0:
                            nc.vector.tensor_copy(dst, po[:])
                        else:
                            nc.scalar.copy(dst, po[:])
                    nc.sync.dma_start(og, ob[:])

    nc.compile()
    return nc


def _get_program():
    if "nc" not in _prog_cache:
        _prog_cache["nc"] = _build_program()
    return _prog_cache["nc"]


def prep_in_maps(x, dist, W1, b1, W2, b2):
    bf16 = ml_dtypes.bfloat16
    x = np.asarray(x, dtype=np.float32)
    dist = np.asarray(dist, dtype=np.float32)
    W1 = np.asarray(W1, dtype=np.float32)
    b1 = np.asarray(b1, dtype=np.float32)
    W2 = np.asarray(W2, dtype=np.float32)
    b2 = np.asarray(b2, dtype=np.float32)

    d = dist / dist.sum()
    dr = np.repeat(d, RB)  # d[p//4] for p=(c,rb)
    # dmat[p, q] = delta(p, q) * d[q//4] / 16: transpose + pool-norm + d
    dmat = np.diag(dr / (P * P)).astype(np.float32)
    # m0[p, m] = delta(p % 4, m)
    m0 = np.tile(np.eye(P, dtype=np.float32), (C, 1)).astype(bf16)
    # t128[p, q] = delta(p, q//4) * d[q//4] / 32 for p < C
    t128 = np.zeros((128, 128), np.float32)
    t128[np.arange(128) // RB, np.arange(128)] = dr / C
    b1r = np.ascontiguousarray(np.broadcast_to(b1, (C, HID))).astype(np.float32)
    w2r = np.ascontiguousarray(np.broadcast_to(W2[0], (C, HID))).astype(np.float32)
    b2r = np.full((C, 1), b2[0], dtype=np.float32)
    w1t = np.ascontiguousarray(W1.T).astype(bf16)

    return [
        dict(
            x=np.ascontiguousarray(x[b]),
            w1t=w1t,
            dmat=dmat,
            m0=m0,
            t128=t128,
            b1r=b1r,
            w2r=w2r,
            b2r=b2r,
        )
        for b in range(N_CORES)
    ]


def kernel(x, dist, W1, b1, W2, b2):
    from concourse.bass_utils import run_bass_kernel_spmd

    in_maps = prep_in_maps(x, dist, W1, b1, W2, b2)
    nc = _get_program()
    res = run_bass_kernel_spmd(nc, in_maps, list(range(N_CORES)))
    out = np.stack([res.results[i]["out"] for i in range(N_CORES)])
    return out[:, None, :, :].astype(np.float32)
